# revision 23
# baseline (speedup 1.0000x reference)
"""Trainium2 Bass kernel for nn_Attention_45148696216373.

8-core data-parallel over tokens (B*S = 131072 -> 16384/core); x is
pre-transposed on the host to [128c, tokens] fp16 so channel sits on
SBUF partitions for the PE matmul.

v2 ("u-path"): instead of folding Wo into a 512-wide per-token VW
projection (which made the DVE combine 1024 MAC/token), contract the
kv axis first in 64-dim space (u[h] = attn[h,0] v0 + attn[h,1] v1,
512 MAC/token) and apply the *shared* Wo via the PE: per 128-token
tile, PE-transpose u -> u2T [c, t'] and matmul against Wo^T. Work is
spread over four engines:

  PE   : 576-wide proj [Q(256)|msum(64)|K0(128)|Vi(128)] + bias rows,
         u transposes (fp16), y = u2 @ Wo^T matmuls
  ACT  : PSUM->SBUF fp16 staging (proj, u2T) + y f32 staging
  DVE  : km = K0*msum, P = q*km, d-fold chain -> attn, C = attn*v
  Pool : k-fold u = C[...,0] + C[...,1] (stride-2 op, 1x on DVE anyway)
  DMA  : x fp16 in (4MB), y f32 out (16MB)

V is staged (d,k)-interleaved so the C-mul runs at DVE 2x with k as
the innermost (step-1) axis of both broadcast operands. All scalar
constants (1/sqrt(D), 1/H) fold into the V columns on the host.

Math (per token t, all ops independent across tokens):
  q_st = x @ Wq^T + bq   -> [D,H] raw-reshaped to [H,D]  (index scramble)
  k_st, v_st similarly -> [KV,D]
  msum = sum_h q[h,:]
  km   = k0 * msum
  attn[h,k] = sum_d q[h,d]*km[k,d]          (scales folded into v')
  u[h,:]  = sum_k attn[h,k]*v'[k,:]         (v' = v/32)
  y[2t+j] = u[2j:2j+2].flat @ Wo^T
"""

import os

# The Bass SPMD path needs the axon trn2 PJRT backend; a cpu pin (e.g. from a
# reference-only harness env) would hide the 8 NeuronCores from jax.devices().
if os.environ.get("JAX_PLATFORMS", "").strip().lower() == "cpu":
    os.environ.pop("JAX_PLATFORMS")

import numpy as np

B, S, DIM = 16, 8192, 128
H, KV, D = 4, 2, 64
T = B * S                 # 131072 tokens
NCORES = 8
TPC = T // NCORES         # 16384 tokens per core
TT = 128                  # tokens per tile (partition dim)
NT = TPC // TT            # 128 tiles per core
GS = 16                   # tiles per group (DVE/ACT op batching)
NG = NT // GS             # 8 groups

NQ = H * D                # 256
NM = D                    # 64
NK = KV * D               # 128
NV = KV * D               # 128 (V block, (d,k)-interleaved cols)
NPROJ = NQ + NM + NK + NV  # 576
OQ, OM, OK, OV = 0, NQ, NQ + NM, NQ + NM + NK

_COMPILED = None


def _fold_weights(Wq, bq, Wk, bk, Wv, bv, Wo):
    """Build W_all [128, 576], bias_all [576], WoT [128, 128] (fp32)."""
    j = np.arange(NQ)
    Wq_p = Wq[j % H, j // H, :]            # [256, 128] col f=(h*64+d)
    bq_p = bq[j % H, j // H]               # [256]
    jk = np.arange(NK)
    Wk_p = Wk[jk % KV, jk // KV, :]        # [128, 128] col f=(k*64+d)
    bk_p = bk[jk % KV, jk // KV]
    Wv_p = Wv[jk % KV, jk // KV, :]        # [128, 128]
    bv_p = bv[jk % KV, jk // KV]

    # msum block: col d = sum_h Wq_p[h*64+d]
    Wm = Wq_p.reshape(H, D, DIM).sum(axis=0)     # [64, 128]
    bm = bq_p.reshape(H, D).sum(axis=0)          # [64]

    # V block, (d,k)-interleaved: col d*2+k = Wv_p[k*64+d] / 32
    # (1/32 = the 1/sqrt(D) attention scale times the 1/H of the q-mean)
    scale = 1.0 / 32.0
    Wv_i = (Wv_p.reshape(KV, D, DIM).transpose(1, 0, 2) * scale).reshape(NV, DIM)
    bv_i = (bv_p.reshape(KV, D).T * scale).reshape(NV)

    W_all = np.concatenate([Wq_p, Wm, Wk_p, Wv_i], axis=0)   # [576, 128]
    b_all = np.concatenate([bq_p, bm, bk_p, bv_i])           # [576]
    return W_all.T.copy(), b_all, Wo.T.copy()


def _numpy_forward(x2d, W_all, b_all, WoT):
    """Host re-implementation of the device math (for validation)."""
    proj = x2d @ W_all + b_all                         # [t, 576]
    Q = proj[:, OQ:OQ + NQ].reshape(-1, H, D)
    msum = proj[:, OM:OM + NM]
    K0 = proj[:, OK:OK + NK].reshape(-1, KV, D)
    Vi = proj[:, OV:OV + NV].reshape(-1, D, KV)        # [t, d, k]
    km = K0 * msum[:, None, :]
    attn = np.einsum("thd,tkd->thk", Q, km)            # [t, 4, 2]
    u = np.einsum("thk,tdk->thd", attn, Vi)            # [t, 4, 64]
    u2 = u.reshape(-1, 2, DIM)                         # [t, j, c]
    y = np.einsum("tjc,co->tjo", u2, WoT)              # [t, j, o]
    return y.reshape(-1, 2 * DIM)                      # [t, 256]


def _build_program():
    import concourse.bass as bass
    import concourse.tile as tile
    from concourse import bacc, mybir

    f32 = mybir.dt.float32
    f16 = mybir.dt.float16
    u64 = mybir.dt.uint64

    nc = bacc.Bacc(
        "TRN2",
        target_bir_lowering=False,
        debug=False,
        enable_asserts=False,
        num_devices=NCORES,
    )

    xT_d = nc.dram_tensor("xT", [DIM, TPC], f16, kind="ExternalInput").ap()
    w_d = nc.dram_tensor("wall", [DIM, NPROJ], f16, kind="ExternalInput").ap()
    b_d = nc.dram_tensor("ball", [1, NPROJ], f16, kind="ExternalInput").ap()
    one_d = nc.dram_tensor("ones", [1, TT], f16, kind="ExternalInput").ap()
    wo_d = nc.dram_tensor("woT", [DIM, DIM], f16, kind="ExternalInput").ap()
    y_d = nc.dram_tensor("y", [TPC, 2 * DIM], f32, kind="ExternalOutput").ap()

    with tile.TileContext(nc) as tc:
        with (
            tc.tile_pool(name="const", bufs=1) as cpool,
            tc.tile_pool(name="xin", bufs=2) as xpool,
            tc.tile_pool(name="pp", bufs=2, space="PSUM") as ppool,
            tc.tile_pool(name="yp", bufs=2, space="PSUM") as yppool,
            tc.tile_pool(name="stg", bufs=2) as spool,
            # DVE-only intermediates: single-buffered (DVE is in-order, so
            # group g+1's writes can never race group g's reads)
            tc.tile_pool(name="work", bufs=1) as wpool,
            tc.tile_pool(name="cc", bufs=2) as cpool2,
            tc.tile_pool(name="u", bufs=2) as upool,
            tc.tile_pool(name="u2t", bufs=2) as u2tpool,
            tc.tile_pool(name="ys", bufs=2) as yspool,
        ):
            w_sb = cpool.tile([DIM, NPROJ], f16)
            nc.sync.dma_start(w_sb[:], w_d[:, :])
            b_sb = cpool.tile([1, NPROJ], f16)
            nc.sync.dma_start(b_sb[:], b_d[:, :])
            one_sb = cpool.tile([1, TT], f16)
            nc.sync.dma_start(one_sb[:], one_d[:, :])
            wo_sb = cpool.tile([DIM, DIM], f16)
            nc.sync.dma_start(wo_sb[:], wo_d[:, :])

            def phase_a(g):
                """Group g: x DMA, projection matmuls, PSUM->SBUF staging."""
                xt = xpool.tile([DIM, GS * TT], f16)
                nc.sync.dma_start(xt[:], xT_d[:, g * GS * TT:(g + 1) * GS * TT])

                stg = spool.tile([TT, GS, NPROJ], f16)
                for i in range(GS):
                    pp = ppool.tile([TT, NPROJ], f32, name=f"pp{i % 2}", tag="pp")
                    xi = xt[:, i * TT:(i + 1) * TT]
                    for lo, hi in ((0, 512), (512, NPROJ)):
                        nc.tensor.matmul(
                            out=pp[:, lo:hi], lhsT=one_sb[:, :],
                            rhs=b_sb[:, lo:hi], start=True, stop=False,
                        )
                        nc.tensor.matmul(
                            out=pp[:, lo:hi], lhsT=xi,
                            rhs=w_sb[:, lo:hi], start=False, stop=True,
                        )
                    nc.scalar.copy(stg[:, i, :], pp[:, :])

                # km[g,k,d] = K0[g,k,d] * msum[g,d] on GPSIMD (flat-rate;
                # Pool has slack). Issued here — before the previous group's
                # kfold in Pool's in-order queue — so it's ready the moment
                # DVE starts this group's P-mul.
                K0 = stg[:, :, OK:OK + NK].rearrange("p g (k d) -> p g k d", k=KV)
                m = stg[:, :, OM:OM + NM]
                km = cpool2.tile([TT, GS, KV, D], f16, name="km")
                nc.gpsimd.tensor_mul(
                    km[:], K0,
                    m.unsqueeze(2).broadcast_to([TT, GS, KV, D]),
                )
                return stg, km

            def phase_b(g, stg, km):
                """Group g: attention math, transposes, Wo matmul, y out."""
                Q = stg[:, :, OQ:OQ + NQ].rearrange("p g (h d) -> p g h d", h=H)
                Vi = stg[:, :, OV:OV + NV].rearrange("p g (d k) -> p g d k", k=KV)

                # P[g,h,k,d] = Q[g,h,d] * km[g,k,d]  (per-h: <=3 free dims)
                P = wpool.tile([TT, GS, H, KV, D], f16)
                for h in range(H):
                    nc.vector.tensor_mul(
                        P[:, :, h],
                        Q[:, :, h].unsqueeze(2).broadcast_to([TT, GS, KV, D]),
                        km[:],
                    )

                # attn[g,h,k] = sum_d P : 2x fold tree over d
                Pf = P[:].rearrange("p g h k d -> p (g h k) d")
                A1 = wpool.tile([TT, GS * 8, 32], f16)
                nc.vector.tensor_add(A1[:], Pf[:, :, 0:32], Pf[:, :, 32:64])
                A2 = wpool.tile([TT, GS * 8, 16], f16)
                nc.vector.tensor_add(A2[:], A1[:, :, 0:16], A1[:, :, 16:32])
                A3 = wpool.tile([TT, GS * 8, 8], f16)
                nc.vector.tensor_add(A3[:], A2[:, :, 0:8], A2[:, :, 8:16])
                A4 = wpool.tile([TT, GS * 8, 4], f16)
                nc.vector.tensor_add(A4[:], A3[:, :, 0:4], A3[:, :, 4:8])
                A5 = wpool.tile([TT, GS * 8, 2], f16)
                nc.vector.tensor_add(A5[:], A4[:, :, 0:2], A4[:, :, 2:4])
                attn = wpool.tile([TT, GS, H, KV], f16)
                nc.vector.tensor_add(
                    attn[:].rearrange("p g h k -> p (g h k)"),
                    A5[:, :, 0], A5[:, :, 1],
                )

                # C[g,h,d,k] = attn[g,h,k] * Vi[g,d,k]  (k innermost: 2x)
                C = cpool2.tile([TT, GS, H, D, KV], f16)
                for h in range(H):
                    nc.vector.tensor_mul(
                        C[:, :, h],
                        attn[:, :, h].unsqueeze(2).broadcast_to([TT, GS, D, KV]),
                        Vi,
                    )

                # Ending pipelined per half-group (8 tiles) so the tail of
                # the last group is short: kfold -> xbar transpose -> y.
                u = upool.tile([TT, GS, 2 * DIM], f16)
                u2t = u2tpool.tile([DIM, GS * 2, TT], f16)
                ys = yspool.tile([TT, GS, 2, DIM], f32)
                HG = GS // 2
                for half in range(2):
                    hs = half * HG
                    # u[g,h,d] = C[...,0] + C[...,1] (stride-2: flat GPSIMD)
                    uv = u[:, hs:hs + HG].rearrange(
                        "p g (h d) -> p (g h) d", h=H)
                    Cf = C[:, hs:hs + HG].rearrange("p g h d k -> p (g h) d k")
                    nc.gpsimd.tensor_add(uv, Cf[:, :, :, 0], Cf[:, :, :, 1])

                    # Batched u transpose through the DMA xbar: one
                    # instruction block-transposes all HG*2 [128,128] blocks
                    # (no PE transposes, no ACT staging).
                    nc.sync.dma_start_transpose(
                        u2t[:, 2 * hs:2 * (hs + HG), :],
                        u[:, hs:hs + HG].rearrange("p g c -> p (g c)"),
                    )

                    # y = u2 @ Wo^T per tile/j; stage PSUM->SBUF per 4 tiles
                    # (bitcast to uint64 halves the ACT element count)
                    for qq in range(HG // 4):
                        q = half * (HG // 4) + qq
                        yp = yppool.tile([TT, 4, 2, DIM], f32, name="yp", tag="yp")
                        for ii in range(4):
                            for j in range(2):
                                nc.tensor.matmul(
                                    out=yp[:, ii, j, :],
                                    lhsT=u2t[:, (q * 4 + ii) * 2 + j, :],
                                    rhs=wo_sb[:, :],
                                    start=True, stop=True,
                                )
                        nc.scalar.copy(ys[:, q * 4:(q + 1) * 4], yp[:])
                # One y DMA per group: SWDGE descriptor-gen cost on the SP
                # sequencer (~1-3us per dma_start) made per-pair DMAs the
                # critical path.
                gbase = g * GS * TT
                dst = y_d[gbase:gbase + GS * TT, :].rearrange(
                    "(i t) (j o) -> t i j o", i=GS, j=2
                )
                nc.sync.dma_start(dst, ys[:])

            # Software pipeline: issue group g+1's projection+staging before
            # group g's tail so in-order engine queues never head-of-line
            # block the next group's critical path.
            nxt = phase_a(0)
            for g in range(NG):
                cur = nxt
                if g + 1 < NG:
                    nxt = phase_a(g + 1)
                phase_b(g, *cur)

    nc.compile()
    return nc


def kernel(x, Wq, bq, Wk, bk, Wv, bv, Wo):
    global _COMPILED
    from concourse.bass_utils import run_bass_kernel_spmd

    x = np.asarray(x, dtype=np.float32)
    W_all, b_all, WoT = _fold_weights(
        np.asarray(Wq, np.float32), np.asarray(bq, np.float32),
        np.asarray(Wk, np.float32), np.asarray(bk, np.float32),
        np.asarray(Wv, np.float32), np.asarray(bv, np.float32),
        np.asarray(Wo, np.float32),
    )

    if _COMPILED is None:
        _COMPILED = _build_program()
    nc = _COMPILED

    x2d = x.reshape(T, DIM)
    ones = np.ones((1, TT), dtype=np.float16)
    wall = W_all.astype(np.float16)
    ball = b_all.reshape(1, NPROJ).astype(np.float16)
    woT = WoT.astype(np.float16)
    in_maps = []
    for c in range(NCORES):
        shard = x2d[c * TPC:(c + 1) * TPC]          # [16384, 128]
        in_maps.append({
            "xT": np.ascontiguousarray(shard.T).astype(np.float16),
            "wall": wall,
            "ball": ball,
            "ones": ones,
            "woT": woT,
        })

    res = run_bass_kernel_spmd(nc, in_maps, list(range(NCORES)))
    ys = [res.results[c]["y"] for c in range(NCORES)]
    Y = np.concatenate(ys, axis=0)                   # [131072, 256]
    return Y.reshape(B, 2 * S, DIM)


# revision 24
# speedup vs baseline: 1.0812x; 1.0812x over previous
"""Trainium2 Bass kernel for nn_Attention_45148696216373.

8-core data-parallel over tokens (B*S = 131072 -> 16384/core); x is
pre-transposed on the host to [128c, tokens] fp16 so channel sits on
SBUF partitions for the PE matmul.

v2 ("u-path"): instead of folding Wo into a 512-wide per-token VW
projection (which made the DVE combine 1024 MAC/token), contract the
kv axis first in 64-dim space (u[h] = attn[h,0] v0 + attn[h,1] v1,
512 MAC/token) and apply the *shared* Wo via the PE: per 128-token
tile, PE-transpose u -> u2T [c, t'] and matmul against Wo^T. Work is
spread over four engines:

  PE   : 576-wide proj [Q(256)|msum(64)|K0(128)|Vi(128)] + bias rows,
         u transposes (fp16), y = u2 @ Wo^T matmuls
  ACT  : PSUM->SBUF fp16 staging (proj, u2T) + y f32 staging
  DVE  : km = K0*msum, P = q*km, d-fold chain -> attn, C = attn*v
  Pool : k-fold u = C[...,0] + C[...,1] (stride-2 op, 1x on DVE anyway)
  DMA  : x fp16 in (4MB), y f32 out (16MB)

V is staged (d,k)-interleaved so the C-mul runs at DVE 2x with k as
the innermost (step-1) axis of both broadcast operands. All scalar
constants (1/sqrt(D), 1/H) fold into the V columns on the host.

Math (per token t, all ops independent across tokens):
  q_st = x @ Wq^T + bq   -> [D,H] raw-reshaped to [H,D]  (index scramble)
  k_st, v_st similarly -> [KV,D]
  msum = sum_h q[h,:]
  km   = k0 * msum
  attn[h,k] = sum_d q[h,d]*km[k,d]          (scales folded into v')
  u[h,:]  = sum_k attn[h,k]*v'[k,:]         (v' = v/32)
  y[2t+j] = u[2j:2j+2].flat @ Wo^T
"""

import os

# The Bass SPMD path needs the axon trn2 PJRT backend; a cpu pin (e.g. from a
# reference-only harness env) would hide the 8 NeuronCores from jax.devices().
if os.environ.get("JAX_PLATFORMS", "").strip().lower() == "cpu":
    os.environ.pop("JAX_PLATFORMS")

import numpy as np

B, S, DIM = 16, 8192, 128
H, KV, D = 4, 2, 64
T = B * S                 # 131072 tokens
NCORES = 8
TPC = T // NCORES         # 16384 tokens per core
TT = 128                  # tokens per tile (partition dim)
NT = TPC // TT            # 128 tiles per core
GS = 16                   # tiles per group (DVE/ACT op batching)
NG = NT // GS             # 8 groups

NQ = H * D                # 256
NM = D                    # 64
NK = KV * D               # 128
NV = KV * D               # 128 (V block, (d,k)-interleaved cols)
NPROJ = NQ + NM + NK + NV  # 576
OQ, OM, OK, OV = 0, NQ, NQ + NM, NQ + NM + NK

_COMPILED = None


def _fold_weights(Wq, bq, Wk, bk, Wv, bv, Wo):
    """Build W_all [128, 576], bias_all [576], WoT [128, 128] (fp32)."""
    j = np.arange(NQ)
    Wq_p = Wq[j % H, j // H, :]            # [256, 128] col f=(h*64+d)
    bq_p = bq[j % H, j // H]               # [256]
    jk = np.arange(NK)
    Wk_p = Wk[jk % KV, jk // KV, :]        # [128, 128] col f=(k*64+d)
    bk_p = bk[jk % KV, jk // KV]
    Wv_p = Wv[jk % KV, jk // KV, :]        # [128, 128]
    bv_p = bv[jk % KV, jk // KV]

    # msum block: col d = sum_h Wq_p[h*64+d]
    Wm = Wq_p.reshape(H, D, DIM).sum(axis=0)     # [64, 128]
    bm = bq_p.reshape(H, D).sum(axis=0)          # [64]

    # V block, (d,k)-interleaved: col d*2+k = Wv_p[k*64+d] / 32
    # (1/32 = the 1/sqrt(D) attention scale times the 1/H of the q-mean)
    scale = 1.0 / 32.0
    Wv_i = (Wv_p.reshape(KV, D, DIM).transpose(1, 0, 2) * scale).reshape(NV, DIM)
    bv_i = (bv_p.reshape(KV, D).T * scale).reshape(NV)

    W_all = np.concatenate([Wq_p, Wm, Wk_p, Wv_i], axis=0)   # [576, 128]
    b_all = np.concatenate([bq_p, bm, bk_p, bv_i])           # [576]
    return W_all.T.copy(), b_all, Wo.T.copy()


def _numpy_forward(x2d, W_all, b_all, WoT):
    """Host re-implementation of the device math (for validation)."""
    proj = x2d @ W_all + b_all                         # [t, 576]
    Q = proj[:, OQ:OQ + NQ].reshape(-1, H, D)
    msum = proj[:, OM:OM + NM]
    K0 = proj[:, OK:OK + NK].reshape(-1, KV, D)
    Vi = proj[:, OV:OV + NV].reshape(-1, D, KV)        # [t, d, k]
    km = K0 * msum[:, None, :]
    attn = np.einsum("thd,tkd->thk", Q, km)            # [t, 4, 2]
    u = np.einsum("thk,tdk->thd", attn, Vi)            # [t, 4, 64]
    u2 = u.reshape(-1, 2, DIM)                         # [t, j, c]
    y = np.einsum("tjc,co->tjo", u2, WoT)              # [t, j, o]
    return y.reshape(-1, 2 * DIM)                      # [t, 256]


def _build_program():
    import concourse.bass as bass
    import concourse.tile as tile
    from concourse import bacc, mybir

    f32 = mybir.dt.float32
    f16 = mybir.dt.float16
    u64 = mybir.dt.uint64

    nc = bacc.Bacc(
        "TRN2",
        target_bir_lowering=False,
        debug=False,
        enable_asserts=False,
        num_devices=NCORES,
    )

    xT_d = nc.dram_tensor("xT", [DIM, TPC], f16, kind="ExternalInput").ap()
    w_d = nc.dram_tensor("wall", [DIM, NPROJ], f16, kind="ExternalInput").ap()
    b_d = nc.dram_tensor("ball", [1, NPROJ], f16, kind="ExternalInput").ap()
    one_d = nc.dram_tensor("ones", [1, TT], f16, kind="ExternalInput").ap()
    wo_d = nc.dram_tensor("woT", [DIM, DIM], f16, kind="ExternalInput").ap()
    y_d = nc.dram_tensor("y", [TPC, 2 * DIM], f32, kind="ExternalOutput").ap()

    with tile.TileContext(nc) as tc:
        with (
            tc.tile_pool(name="const", bufs=1) as cpool,
            tc.tile_pool(name="xin", bufs=2) as xpool,
            tc.tile_pool(name="pp", bufs=2, space="PSUM") as ppool,
            tc.tile_pool(name="yp", bufs=2, space="PSUM") as yppool,
            tc.tile_pool(name="stg", bufs=3) as spool,
            # DVE-only intermediates: single-buffered (DVE is in-order, so
            # group g+1's writes can never race group g's reads)
            tc.tile_pool(name="work", bufs=1) as wpool,
            tc.tile_pool(name="cc", bufs=2) as cpool2,
            tc.tile_pool(name="km", bufs=3) as kmpool,
            tc.tile_pool(name="u", bufs=2) as upool,
            tc.tile_pool(name="u2t", bufs=2) as u2tpool,
            tc.tile_pool(name="ys", bufs=1) as yspool,
        ):
            w_sb = cpool.tile([DIM, NPROJ], f16)
            nc.sync.dma_start(w_sb[:], w_d[:, :])
            b_sb = cpool.tile([1, NPROJ], f16)
            nc.sync.dma_start(b_sb[:], b_d[:, :])
            one_sb = cpool.tile([1, TT], f16)
            nc.sync.dma_start(one_sb[:], one_d[:, :])
            wo_sb = cpool.tile([DIM, DIM], f16)
            nc.sync.dma_start(wo_sb[:], wo_d[:, :])

            def phase_a(g):
                """Group g: x DMA, projection matmuls, PSUM->SBUF staging."""
                xt = xpool.tile([DIM, GS * TT], f16)
                nc.sync.dma_start(xt[:], xT_d[:, g * GS * TT:(g + 1) * GS * TT])

                stg = spool.tile([TT, GS, NPROJ], f16)
                for i in range(GS):
                    pp = ppool.tile([TT, NPROJ], f32, name=f"pp{i % 2}", tag="pp")
                    xi = xt[:, i * TT:(i + 1) * TT]
                    for lo, hi in ((0, 512), (512, NPROJ)):
                        nc.tensor.matmul(
                            out=pp[:, lo:hi], lhsT=one_sb[:, :],
                            rhs=b_sb[:, lo:hi], start=True, stop=False,
                        )
                        nc.tensor.matmul(
                            out=pp[:, lo:hi], lhsT=xi,
                            rhs=w_sb[:, lo:hi], start=False, stop=True,
                        )
                    nc.scalar.copy(stg[:, i, :], pp[:, :])

                # km[g,k,d] = K0[g,k,d] * msum[g,d] on GPSIMD (flat-rate;
                # Pool has slack). Issued here — before the previous group's
                # kfold in Pool's in-order queue — so it's ready the moment
                # DVE starts this group's P-mul.
                K0 = stg[:, :, OK:OK + NK].rearrange("p g (k d) -> p g k d", k=KV)
                m = stg[:, :, OM:OM + NM]
                km = kmpool.tile([TT, GS, KV, D], f16, name="km")
                nc.gpsimd.tensor_mul(
                    km[:], K0,
                    m.unsqueeze(2).broadcast_to([TT, GS, KV, D]),
                )
                return stg, km

            def phase_b1(g, stg, km):
                """Group g DVE math: P, fold tree -> attn, C."""
                Q = stg[:, :, OQ:OQ + NQ].rearrange("p g (h d) -> p g h d", h=H)
                Vi = stg[:, :, OV:OV + NV].rearrange("p g (d k) -> p g d k", k=KV)

                # P[g,h,k,d] = Q[g,h,d] * km[g,k,d]  (per-h: <=3 free dims)
                P = wpool.tile([TT, GS, H, KV, D], f16)
                for h in range(H):
                    nc.vector.tensor_mul(
                        P[:, :, h],
                        Q[:, :, h].unsqueeze(2).broadcast_to([TT, GS, KV, D]),
                        km[:],
                    )

                # attn[g,h,k] = sum_d P : 2x fold tree over d
                Pf = P[:].rearrange("p g h k d -> p (g h k) d")
                A1 = wpool.tile([TT, GS * 8, 32], f16)
                nc.vector.tensor_add(A1[:], Pf[:, :, 0:32], Pf[:, :, 32:64])
                A2 = wpool.tile([TT, GS * 8, 16], f16)
                nc.vector.tensor_add(A2[:], A1[:, :, 0:16], A1[:, :, 16:32])
                A3 = wpool.tile([TT, GS * 8, 8], f16)
                nc.vector.tensor_add(A3[:], A2[:, :, 0:8], A2[:, :, 8:16])
                A4 = wpool.tile([TT, GS * 8, 4], f16)
                nc.vector.tensor_add(A4[:], A3[:, :, 0:4], A3[:, :, 4:8])
                A5 = wpool.tile([TT, GS * 8, 2], f16)
                nc.vector.tensor_add(A5[:], A4[:, :, 0:2], A4[:, :, 2:4])
                attn = wpool.tile([TT, GS, H, KV], f16)
                nc.vector.tensor_add(
                    attn[:].rearrange("p g h k -> p (g h k)"),
                    A5[:, :, 0], A5[:, :, 1],
                )

                # C[g,h,d,k] = attn[g,h,k] * Vi[g,d,k]  (k innermost: 2x)
                C = cpool2.tile([TT, GS, H, D, KV], f16)
                for h in range(H):
                    nc.vector.tensor_mul(
                        C[:, :, h],
                        attn[:, :, h].unsqueeze(2).broadcast_to([TT, GS, D, KV]),
                        Vi,
                    )

                return C

            def phase_b2(g, C):
                """Group g ending: kfold -> xbar transpose -> y = u2@Wo^T.
                Deferred one pipeline step behind phase_b1 so these tail ops
                sit *after* the next groups' staging in every in-order
                engine queue (no head-of-line blocking of the critical
                DVE path)."""
                u = upool.tile([TT, GS, 2 * DIM], f16)
                u2t = u2tpool.tile([DIM, GS * 2, TT], f16)
                ys = yspool.tile([TT, GS, 2, DIM], f32)
                HG = GS // 2
                for half in range(2):
                    hs = half * HG
                    # u[g,h,d] = C[...,0] + C[...,1] (stride-2: flat GPSIMD)
                    uv = u[:, hs:hs + HG].rearrange(
                        "p g (h d) -> p (g h) d", h=H)
                    Cf = C[:, hs:hs + HG].rearrange("p g h d k -> p (g h) d k")
                    nc.gpsimd.tensor_add(uv, Cf[:, :, :, 0], Cf[:, :, :, 1])

                    # Batched u transpose through the DMA xbar: one
                    # instruction block-transposes all HG*2 [128,128] blocks
                    # (no PE transposes, no ACT staging).
                    nc.sync.dma_start_transpose(
                        u2t[:, 2 * hs:2 * (hs + HG), :],
                        u[:, hs:hs + HG].rearrange("p g c -> p (g c)"),
                    )

                    # y = u2 @ Wo^T per tile/j; stage PSUM->SBUF per 4 tiles
                    # (bitcast to uint64 halves the ACT element count)
                    for qq in range(HG // 4):
                        q = half * (HG // 4) + qq
                        yp = yppool.tile([TT, 4, 2, DIM], f32, name="yp", tag="yp")
                        for ii in range(4):
                            for j in range(2):
                                nc.tensor.matmul(
                                    out=yp[:, ii, j, :],
                                    lhsT=u2t[:, (q * 4 + ii) * 2 + j, :],
                                    rhs=wo_sb[:, :],
                                    start=True, stop=True,
                                )
                        nc.scalar.copy(ys[:, q * 4:(q + 1) * 4], yp[:])
                # One y DMA per group: SWDGE descriptor-gen cost on the SP
                # sequencer (~1-3us per dma_start) made per-pair DMAs the
                # critical path.
                gbase = g * GS * TT
                dst = y_d[gbase:gbase + GS * TT, :].rearrange(
                    "(i t) (j o) -> t i j o", i=GS, j=2
                )
                nc.sync.dma_start(dst, ys[:])

            # Software pipeline: issue group g+1's projection+staging before
            # group g's tail so in-order engine queues never head-of-line
            # block the next group's critical path.
            nxt = phase_a(0)
            prev_c = None
            for g in range(NG):
                cur = nxt
                if g + 1 < NG:
                    nxt = phase_a(g + 1)
                c = phase_b1(g, *cur)
                if prev_c is not None:
                    phase_b2(g - 1, prev_c)
                prev_c = c
            phase_b2(NG - 1, prev_c)

    nc.compile()
    return nc


def kernel(x, Wq, bq, Wk, bk, Wv, bv, Wo):
    global _COMPILED
    from concourse.bass_utils import run_bass_kernel_spmd

    x = np.asarray(x, dtype=np.float32)
    W_all, b_all, WoT = _fold_weights(
        np.asarray(Wq, np.float32), np.asarray(bq, np.float32),
        np.asarray(Wk, np.float32), np.asarray(bk, np.float32),
        np.asarray(Wv, np.float32), np.asarray(bv, np.float32),
        np.asarray(Wo, np.float32),
    )

    if _COMPILED is None:
        _COMPILED = _build_program()
    nc = _COMPILED

    x2d = x.reshape(T, DIM)
    ones = np.ones((1, TT), dtype=np.float16)
    wall = W_all.astype(np.float16)
    ball = b_all.reshape(1, NPROJ).astype(np.float16)
    woT = WoT.astype(np.float16)
    in_maps = []
    for c in range(NCORES):
        shard = x2d[c * TPC:(c + 1) * TPC]          # [16384, 128]
        in_maps.append({
            "xT": np.ascontiguousarray(shard.T).astype(np.float16),
            "wall": wall,
            "ball": ball,
            "ones": ones,
            "woT": woT,
        })

    res = run_bass_kernel_spmd(nc, in_maps, list(range(NCORES)))
    ys = [res.results[c]["y"] for c in range(NCORES)]
    Y = np.concatenate(ys, axis=0)                   # [131072, 256]
    return Y.reshape(B, 2 * S, DIM)


# revision 26
# speedup vs baseline: 1.2017x; 1.1114x over previous
"""Trainium2 Bass kernel for nn_Attention_45148696216373.

8-core data-parallel over tokens (B*S = 131072 -> 16384/core); x is
pre-transposed on the host to [128c, tokens] fp16 so channel sits on
SBUF partitions for the PE matmul.

v2 ("u-path"): instead of folding Wo into a 512-wide per-token VW
projection (which made the DVE combine 1024 MAC/token), contract the
kv axis first in 64-dim space (u[h] = attn[h,0] v0 + attn[h,1] v1,
512 MAC/token) and apply the *shared* Wo via the PE: per 128-token
tile, PE-transpose u -> u2T [c, t'] and matmul against Wo^T. Work is
spread over four engines:

  PE   : 576-wide proj [Q(256)|msum(64)|K0(128)|Vi(128)] + bias rows,
         u transposes (fp16), y = u2 @ Wo^T matmuls
  ACT  : PSUM->SBUF fp16 staging (proj, u2T) + y f32 staging
  DVE  : km = K0*msum, P = q*km, d-fold chain -> attn, C = attn*v
  Pool : k-fold u = C[...,0] + C[...,1] (stride-2 op, 1x on DVE anyway)
  DMA  : x fp16 in (4MB), y f32 out (16MB)

V is staged (d,k)-interleaved so the C-mul runs at DVE 2x with k as
the innermost (step-1) axis of both broadcast operands. All scalar
constants (1/sqrt(D), 1/H) fold into the V columns on the host.

Math (per token t, all ops independent across tokens):
  q_st = x @ Wq^T + bq   -> [D,H] raw-reshaped to [H,D]  (index scramble)
  k_st, v_st similarly -> [KV,D]
  msum = sum_h q[h,:]
  km   = k0 * msum
  attn[h,k] = sum_d q[h,d]*km[k,d]          (scales folded into v')
  u[h,:]  = sum_k attn[h,k]*v'[k,:]         (v' = v/32)
  y[2t+j] = u[2j:2j+2].flat @ Wo^T
"""

import os

# The Bass SPMD path needs the axon trn2 PJRT backend; a cpu pin (e.g. from a
# reference-only harness env) would hide the 8 NeuronCores from jax.devices().
if os.environ.get("JAX_PLATFORMS", "").strip().lower() == "cpu":
    os.environ.pop("JAX_PLATFORMS")

import numpy as np

B, S, DIM = 16, 8192, 128
H, KV, D = 4, 2, 64
T = B * S                 # 131072 tokens
NCORES = 8
TPC = T // NCORES         # 16384 tokens per core
TT = 128                  # tokens per tile (partition dim)
NT = TPC // TT            # 128 tiles per core
GS = 16                   # tiles per group (DVE/ACT op batching)
NG = NT // GS             # 8 groups

NQ = H * D                # 256
NM = D                    # 64
NK = KV * D               # 128
NV = KV * D               # 128 (V block, (d,k)-interleaved cols)
NPROJ = NQ + NM + NK + NV  # 576
OQ, OM, OK, OV = 0, NQ, NQ + NM, NQ + NM + NK

_COMPILED = None


def _fold_weights(Wq, bq, Wk, bk, Wv, bv, Wo):
    """Build W_all [128, 576], bias_all [576], WoT [128, 128] (fp32)."""
    j = np.arange(NQ)
    Wq_p = Wq[j % H, j // H, :]            # [256, 128] col f=(h*64+d)
    bq_p = bq[j % H, j // H]               # [256]
    jk = np.arange(NK)
    Wk_p = Wk[jk % KV, jk // KV, :]        # [128, 128] col f=(k*64+d)
    bk_p = bk[jk % KV, jk // KV]
    Wv_p = Wv[jk % KV, jk // KV, :]        # [128, 128]
    bv_p = bv[jk % KV, jk // KV]

    # msum block: col d = sum_h Wq_p[h*64+d]
    Wm = Wq_p.reshape(H, D, DIM).sum(axis=0)     # [64, 128]
    bm = bq_p.reshape(H, D).sum(axis=0)          # [64]

    # V block, (d,k)-interleaved: col d*2+k = Wv_p[k*64+d] / 32
    # (1/32 = the 1/sqrt(D) attention scale times the 1/H of the q-mean)
    scale = 1.0 / 32.0
    Wv_i = (Wv_p.reshape(KV, D, DIM).transpose(1, 0, 2) * scale).reshape(NV, DIM)
    bv_i = (bv_p.reshape(KV, D).T * scale).reshape(NV)

    W_all = np.concatenate([Wq_p, Wm, Wk_p, Wv_i], axis=0)   # [576, 128]
    b_all = np.concatenate([bq_p, bm, bk_p, bv_i])           # [576]
    return W_all.T.copy(), b_all, Wo.T.copy()


def _numpy_forward(x2d, W_all, b_all, WoT):
    """Host re-implementation of the device math (for validation)."""
    proj = x2d @ W_all + b_all                         # [t, 576]
    Q = proj[:, OQ:OQ + NQ].reshape(-1, H, D)
    msum = proj[:, OM:OM + NM]
    K0 = proj[:, OK:OK + NK].reshape(-1, KV, D)
    Vi = proj[:, OV:OV + NV].reshape(-1, D, KV)        # [t, d, k]
    km = K0 * msum[:, None, :]
    attn = np.einsum("thd,tkd->thk", Q, km)            # [t, 4, 2]
    u = np.einsum("thk,tdk->thd", attn, Vi)            # [t, 4, 64]
    u2 = u.reshape(-1, 2, DIM)                         # [t, j, c]
    y = np.einsum("tjc,co->tjo", u2, WoT)              # [t, j, o]
    return y.reshape(-1, 2 * DIM)                      # [t, 256]


def _build_program():
    import concourse.bass as bass
    import concourse.tile as tile
    from concourse import bacc, mybir

    f32 = mybir.dt.float32
    f16 = mybir.dt.float16
    u64 = mybir.dt.uint64

    nc = bacc.Bacc(
        "TRN2",
        target_bir_lowering=False,
        debug=False,
        enable_asserts=False,
        num_devices=NCORES,
    )

    xT_d = nc.dram_tensor("xT", [DIM, TPC], f16, kind="ExternalInput").ap()
    w_d = nc.dram_tensor("wall", [DIM, NPROJ], f16, kind="ExternalInput").ap()
    b_d = nc.dram_tensor("ball", [1, NPROJ], f16, kind="ExternalInput").ap()
    one_d = nc.dram_tensor("ones", [1, TT], f16, kind="ExternalInput").ap()
    wo_d = nc.dram_tensor("woT", [DIM, DIM], f16, kind="ExternalInput").ap()
    y_d = nc.dram_tensor("y", [TPC, 2 * DIM], f32, kind="ExternalOutput").ap()

    with tile.TileContext(nc) as tc:
        with (
            tc.tile_pool(name="const", bufs=1) as cpool,
            tc.tile_pool(name="xin", bufs=2) as xpool,
            tc.tile_pool(name="pp", bufs=2, space="PSUM") as ppool,
            tc.tile_pool(name="yp", bufs=2, space="PSUM") as yppool,
            tc.tile_pool(name="stg", bufs=3) as spool,
            # DVE-only intermediates: single-buffered (DVE is in-order, so
            # group g+1's writes can never race group g's reads)
            tc.tile_pool(name="work", bufs=1) as wpool,
            tc.tile_pool(name="cc", bufs=2) as cpool2,
            tc.tile_pool(name="km", bufs=3) as kmpool,
            tc.tile_pool(name="u", bufs=2) as upool,
            tc.tile_pool(name="u2t", bufs=2) as u2tpool,
            tc.tile_pool(name="ys", bufs=1) as yspool,
        ):
            w_sb = cpool.tile([DIM, NPROJ], f16)
            nc.sync.dma_start(w_sb[:], w_d[:, :])
            b_sb = cpool.tile([1, NPROJ], f16)
            nc.sync.dma_start(b_sb[:], b_d[:, :])
            one_sb = cpool.tile([1, TT], f16)
            nc.sync.dma_start(one_sb[:], one_d[:, :])
            wo_sb = cpool.tile([DIM, DIM], f16)
            nc.sync.dma_start(wo_sb[:], wo_d[:, :])

            def phase_a(g):
                """Group g: x DMA, projection matmuls, PSUM->SBUF staging."""
                xt = xpool.tile([DIM, GS * TT], f16)
                nc.sync.dma_start(xt[:], xT_d[:, g * GS * TT:(g + 1) * GS * TT])

                stg = spool.tile([TT, GS, NPROJ], f16)
                for i in range(GS):
                    pp = ppool.tile([TT, NPROJ], f32, name=f"pp{i % 2}", tag="pp")
                    xi = xt[:, i * TT:(i + 1) * TT]
                    for lo, hi in ((0, 512), (512, NPROJ)):
                        nc.tensor.matmul(
                            out=pp[:, lo:hi], lhsT=one_sb[:, :],
                            rhs=b_sb[:, lo:hi], start=True, stop=False,
                        )
                        nc.tensor.matmul(
                            out=pp[:, lo:hi], lhsT=xi,
                            rhs=w_sb[:, lo:hi], start=False, stop=True,
                        )
                    nc.scalar.copy(stg[:, i, :], pp[:, :])

                # km[g,k,d] = K0[g,k,d] * msum[g,d] on GPSIMD (flat-rate;
                # Pool has slack). Issued here — before the previous group's
                # kfold in Pool's in-order queue — so it's ready the moment
                # DVE starts this group's P-mul.
                K0 = stg[:, :, OK:OK + NK].rearrange("p g (k d) -> p g k d", k=KV)
                m = stg[:, :, OM:OM + NM]
                km = kmpool.tile([TT, GS, KV, D], f16, name="km")
                nc.gpsimd.tensor_mul(
                    km[:], K0,
                    m.unsqueeze(2).broadcast_to([TT, GS, KV, D]),
                )
                return stg, km

            def phase_b1(g, stg, km):
                """Group g DVE math: P, fold tree -> attn, C."""
                Q = stg[:, :, OQ:OQ + NQ].rearrange("p g (h d) -> p g h d", h=H)
                Vi = stg[:, :, OV:OV + NV].rearrange("p g (d k) -> p g d k", k=KV)

                # P[g,h,k,d] = Q[g,h,d] * km[g,k,d]  (per-h: <=3 free dims)
                P = wpool.tile([TT, GS, H, KV, D], f16)
                for h in range(H):
                    nc.vector.tensor_mul(
                        P[:, :, h],
                        Q[:, :, h].unsqueeze(2).broadcast_to([TT, GS, KV, D]),
                        km[:],
                    )

                # attn[g,h,k] = sum_d P : 2x fold tree over d
                Pf = P[:].rearrange("p g h k d -> p (g h k) d")
                A1 = wpool.tile([TT, GS * 8, 32], f16)
                nc.vector.tensor_add(A1[:], Pf[:, :, 0:32], Pf[:, :, 32:64])
                A2 = wpool.tile([TT, GS * 8, 16], f16)
                nc.vector.tensor_add(A2[:], A1[:, :, 0:16], A1[:, :, 16:32])
                A3 = wpool.tile([TT, GS * 8, 8], f16)
                nc.vector.tensor_add(A3[:], A2[:, :, 0:8], A2[:, :, 8:16])
                A4 = wpool.tile([TT, GS * 8, 4], f16)
                nc.vector.tensor_add(A4[:], A3[:, :, 0:4], A3[:, :, 4:8])
                A5 = wpool.tile([TT, GS * 8, 2], f16)
                nc.vector.tensor_add(A5[:], A4[:, :, 0:2], A4[:, :, 2:4])
                attn = wpool.tile([TT, GS, H, KV], f16)
                nc.vector.tensor_add(
                    attn[:].rearrange("p g h k -> p (g h k)"),
                    A5[:, :, 0], A5[:, :, 1],
                )

                # C[g,h,d,k] = attn[g,h,k] * Vi[g,d,k]  (k innermost: 2x;
                # issued per half-group so the ending can start early)
                C = cpool2.tile([TT, GS, H, D, KV], f16)
                for hf in range(2):
                    sl = slice(hf * (GS // 2), (hf + 1) * (GS // 2))
                    for h in range(H):
                        nc.vector.tensor_mul(
                            C[:, sl, h],
                            attn[:, sl, h].unsqueeze(2).broadcast_to(
                                [TT, GS // 2, D, KV]),
                            Vi[:, sl],
                        )

                return C

            def phase_b2(g, C):
                """Group g ending: kfold -> xbar transpose -> y = u2@Wo^T.
                Deferred one pipeline step behind phase_b1 so these tail ops
                sit *after* the next groups' staging in every in-order
                engine queue (no head-of-line blocking of the critical
                DVE path)."""
                u = upool.tile([TT, GS, 2 * DIM], f16)
                u2t = u2tpool.tile([DIM, GS * 2, TT], f16)
                ys = yspool.tile([TT, GS, 2, DIM], f32)
                HG = GS // 2
                for half in range(2):
                    hs = half * HG
                    # u[g,h,d] = C[...,0] + C[...,1] (stride-2: flat GPSIMD)
                    uv = u[:, hs:hs + HG].rearrange(
                        "p g (h d) -> p (g h) d", h=H)
                    Cf = C[:, hs:hs + HG].rearrange("p g h d k -> p (g h) d k")
                    nc.gpsimd.tensor_add(uv, Cf[:, :, :, 0], Cf[:, :, :, 1])

                    # Batched u transpose through the DMA xbar: one
                    # instruction block-transposes all HG*2 [128,128] blocks
                    # (no PE transposes, no ACT staging).
                    nc.sync.dma_start_transpose(
                        u2t[:, 2 * hs:2 * (hs + HG), :],
                        u[:, hs:hs + HG].rearrange("p g c -> p (g c)"),
                    )

                    # y = u2 @ Wo^T per tile/j; stage PSUM->SBUF per 4 tiles
                    # (bitcast to uint64 halves the ACT element count)
                    for qq in range(HG // 4):
                        q = half * (HG // 4) + qq
                        yp = yppool.tile([TT, 4, 2, DIM], f32, name="yp", tag="yp")
                        for ii in range(4):
                            for j in range(2):
                                nc.tensor.matmul(
                                    out=yp[:, ii, j, :],
                                    lhsT=u2t[:, (q * 4 + ii) * 2 + j, :],
                                    rhs=wo_sb[:, :],
                                    start=True, stop=True,
                                )
                        nc.scalar.copy(ys[:, q * 4:(q + 1) * 4], yp[:])
                    hbase = (g * GS + hs) * TT
                    dst = y_d[hbase:hbase + HG * TT, :].rearrange(
                        "(i t) (j o) -> t i j o", i=HG, j=2
                    )
                    nc.sync.dma_start(dst, ys[:, hs:hs + HG])

            # Software pipeline: issue group g+1's projection+staging before
            # group g's tail so in-order engine queues never head-of-line
            # block the next group's critical path.
            nxt = phase_a(0)
            prev_c = None
            for g in range(NG):
                cur = nxt
                if g + 1 < NG:
                    nxt = phase_a(g + 1)
                c = phase_b1(g, *cur)
                if prev_c is not None:
                    phase_b2(g - 1, prev_c)
                prev_c = c
            phase_b2(NG - 1, prev_c)

    nc.compile()
    return nc


def kernel(x, Wq, bq, Wk, bk, Wv, bv, Wo):
    global _COMPILED
    from concourse.bass_utils import run_bass_kernel_spmd

    x = np.asarray(x, dtype=np.float32)
    W_all, b_all, WoT = _fold_weights(
        np.asarray(Wq, np.float32), np.asarray(bq, np.float32),
        np.asarray(Wk, np.float32), np.asarray(bk, np.float32),
        np.asarray(Wv, np.float32), np.asarray(bv, np.float32),
        np.asarray(Wo, np.float32),
    )

    if _COMPILED is None:
        _COMPILED = _build_program()
    nc = _COMPILED

    x2d = x.reshape(T, DIM)
    ones = np.ones((1, TT), dtype=np.float16)
    wall = W_all.astype(np.float16)
    ball = b_all.reshape(1, NPROJ).astype(np.float16)
    woT = WoT.astype(np.float16)
    in_maps = []
    for c in range(NCORES):
        shard = x2d[c * TPC:(c + 1) * TPC]          # [16384, 128]
        in_maps.append({
            "xT": np.ascontiguousarray(shard.T).astype(np.float16),
            "wall": wall,
            "ball": ball,
            "ones": ones,
            "woT": woT,
        })

    res = run_bass_kernel_spmd(nc, in_maps, list(range(NCORES)))
    ys = [res.results[c]["y"] for c in range(NCORES)]
    Y = np.concatenate(ys, axis=0)                   # [131072, 256]
    return Y.reshape(B, 2 * S, DIM)


# revision 28
# speedup vs baseline: 1.2590x; 1.0477x over previous
"""Trainium2 Bass kernel for nn_Attention_45148696216373.

8-core data-parallel over tokens (B*S = 131072 -> 16384/core); x is
pre-transposed on the host to [128c, tokens] fp16 so channel sits on
SBUF partitions for the PE matmul.

v2 ("u-path"): instead of folding Wo into a 512-wide per-token VW
projection (which made the DVE combine 1024 MAC/token), contract the
kv axis first in 64-dim space (u[h] = attn[h,0] v0 + attn[h,1] v1,
512 MAC/token) and apply the *shared* Wo via the PE: per 128-token
tile, PE-transpose u -> u2T [c, t'] and matmul against Wo^T. Work is
spread over four engines:

  PE   : 576-wide proj [Q(256)|msum(64)|K0(128)|Vi(128)] + bias rows,
         u transposes (fp16), y = u2 @ Wo^T matmuls
  ACT  : PSUM->SBUF fp16 staging (proj, u2T) + y f32 staging
  DVE  : km = K0*msum, P = q*km, d-fold chain -> attn, C = attn*v
  Pool : k-fold u = C[...,0] + C[...,1] (stride-2 op, 1x on DVE anyway)
  DMA  : x fp16 in (4MB), y f32 out (16MB)

V is staged (d,k)-interleaved so the C-mul runs at DVE 2x with k as
the innermost (step-1) axis of both broadcast operands. All scalar
constants (1/sqrt(D), 1/H) fold into the V columns on the host.

Math (per token t, all ops independent across tokens):
  q_st = x @ Wq^T + bq   -> [D,H] raw-reshaped to [H,D]  (index scramble)
  k_st, v_st similarly -> [KV,D]
  msum = sum_h q[h,:]
  km   = k0 * msum
  attn[h,k] = sum_d q[h,d]*km[k,d]          (scales folded into v')
  u[h,:]  = sum_k attn[h,k]*v'[k,:]         (v' = v/32)
  y[2t+j] = u[2j:2j+2].flat @ Wo^T
"""

import os

# The Bass SPMD path needs the axon trn2 PJRT backend; a cpu pin (e.g. from a
# reference-only harness env) would hide the 8 NeuronCores from jax.devices().
if os.environ.get("JAX_PLATFORMS", "").strip().lower() == "cpu":
    os.environ.pop("JAX_PLATFORMS")

import numpy as np

B, S, DIM = 16, 8192, 128
H, KV, D = 4, 2, 64
T = B * S                 # 131072 tokens
NCORES = 8
TPC = T // NCORES         # 16384 tokens per core
TT = 128                  # tokens per tile (partition dim)
NT = TPC // TT            # 128 tiles per core
GS = 16                   # tiles per group (DVE/ACT op batching)
NG = NT // GS             # 8 groups

NQ = H * D                # 256
NM = D                    # 64
NK = KV * D               # 128
NV = KV * D               # 128 (V block, (d,k)-interleaved cols)
NPROJ = NQ + NM + NK + NV  # 576
OQ, OM, OK, OV = 0, NQ, NQ + NM, NQ + NM + NK

_COMPILED = None


def _fold_weights(Wq, bq, Wk, bk, Wv, bv, Wo):
    """Build W_all [128, 576], bias_all [576], WoT [128, 128] (fp32)."""
    j = np.arange(NQ)
    Wq_p = Wq[j % H, j // H, :]            # [256, 128] col f=(h*64+d)
    bq_p = bq[j % H, j // H]               # [256]
    jk = np.arange(NK)
    Wk_p = Wk[jk % KV, jk // KV, :]        # [128, 128] col f=(k*64+d)
    bk_p = bk[jk % KV, jk // KV]
    Wv_p = Wv[jk % KV, jk // KV, :]        # [128, 128]
    bv_p = bv[jk % KV, jk // KV]

    # msum block: col d = sum_h Wq_p[h*64+d]
    Wm = Wq_p.reshape(H, D, DIM).sum(axis=0)     # [64, 128]
    bm = bq_p.reshape(H, D).sum(axis=0)          # [64]

    # V block, (d,k)-interleaved: col d*2+k = Wv_p[k*64+d] / 32
    # (1/32 = the 1/sqrt(D) attention scale times the 1/H of the q-mean)
    scale = 1.0 / 32.0
    Wv_i = (Wv_p.reshape(KV, D, DIM).transpose(1, 0, 2) * scale).reshape(NV, DIM)
    bv_i = (bv_p.reshape(KV, D).T * scale).reshape(NV)

    W_all = np.concatenate([Wq_p, Wm, Wk_p, Wv_i], axis=0)   # [576, 128]
    b_all = np.concatenate([bq_p, bm, bk_p, bv_i])           # [576]
    return W_all.T.copy(), b_all, Wo.T.copy()


def _numpy_forward(x2d, W_all, b_all, WoT):
    """Host re-implementation of the device math (for validation)."""
    proj = x2d @ W_all + b_all                         # [t, 576]
    Q = proj[:, OQ:OQ + NQ].reshape(-1, H, D)
    msum = proj[:, OM:OM + NM]
    K0 = proj[:, OK:OK + NK].reshape(-1, KV, D)
    Vi = proj[:, OV:OV + NV].reshape(-1, D, KV)        # [t, d, k]
    km = K0 * msum[:, None, :]
    attn = np.einsum("thd,tkd->thk", Q, km)            # [t, 4, 2]
    u = np.einsum("thk,tdk->thd", attn, Vi)            # [t, 4, 64]
    u2 = u.reshape(-1, 2, DIM)                         # [t, j, c]
    y = np.einsum("tjc,co->tjo", u2, WoT)              # [t, j, o]
    return y.reshape(-1, 2 * DIM)                      # [t, 256]


def _build_program():
    import concourse.bass as bass
    import concourse.tile as tile
    from concourse import bacc, mybir

    f32 = mybir.dt.float32
    f16 = mybir.dt.float16
    u64 = mybir.dt.uint64

    nc = bacc.Bacc(
        "TRN2",
        target_bir_lowering=False,
        debug=False,
        enable_asserts=False,
        num_devices=NCORES,
    )

    xT_d = nc.dram_tensor("xT", [DIM, TPC], f16, kind="ExternalInput").ap()
    w_d = nc.dram_tensor("wall", [DIM, NPROJ], f16, kind="ExternalInput").ap()
    b_d = nc.dram_tensor("ball", [1, NPROJ], f16, kind="ExternalInput").ap()
    one_d = nc.dram_tensor("ones", [1, TT], f16, kind="ExternalInput").ap()
    wo_d = nc.dram_tensor("woT", [DIM, DIM], f16, kind="ExternalInput").ap()
    y_d = nc.dram_tensor("y", [TPC, 2 * DIM], f32, kind="ExternalOutput").ap()

    with tile.TileContext(nc) as tc:
        with (
            tc.tile_pool(name="const", bufs=1) as cpool,
            tc.tile_pool(name="xin", bufs=2) as xpool,
            tc.tile_pool(name="pp", bufs=2, space="PSUM") as ppool,
            tc.tile_pool(name="yp", bufs=2, space="PSUM") as yppool,
            tc.tile_pool(name="stg", bufs=3) as spool,
            # DVE-only intermediates: single-buffered (DVE is in-order, so
            # group g+1's writes can never race group g's reads)
            tc.tile_pool(name="work", bufs=1) as wpool,
            tc.tile_pool(name="cc", bufs=2) as cpool2,
            tc.tile_pool(name="km", bufs=3) as kmpool,
            tc.tile_pool(name="u", bufs=2) as upool,
            tc.tile_pool(name="u2t", bufs=2) as u2tpool,
            tc.tile_pool(name="ys", bufs=1) as yspool,
        ):
            w_sb = cpool.tile([DIM, NPROJ], f16)
            nc.sync.dma_start(w_sb[:], w_d[:, :])
            b_sb = cpool.tile([1, NPROJ], f16)
            nc.sync.dma_start(b_sb[:], b_d[:, :])
            one_sb = cpool.tile([1, TT], f16)
            nc.sync.dma_start(one_sb[:], one_d[:, :])
            wo_sb = cpool.tile([DIM, DIM], f16)
            nc.sync.dma_start(wo_sb[:], wo_d[:, :])

            def phase_a(g, emit_km=True):
                """Group g: x DMA, projection matmuls, PSUM->SBUF staging."""
                xt = xpool.tile([DIM, GS * TT], f16)
                nc.sync.dma_start(xt[:], xT_d[:, g * GS * TT:(g + 1) * GS * TT])

                stg = spool.tile([TT, GS, NPROJ], f16)
                for i in range(GS):
                    pp = ppool.tile([TT, NPROJ], f32, name=f"pp{i % 2}", tag="pp")
                    xi = xt[:, i * TT:(i + 1) * TT]
                    for lo, hi in ((0, 512), (512, NPROJ)):
                        nc.tensor.matmul(
                            out=pp[:, lo:hi], lhsT=one_sb[:, :],
                            rhs=b_sb[:, lo:hi], start=True, stop=False,
                        )
                        nc.tensor.matmul(
                            out=pp[:, lo:hi], lhsT=xi,
                            rhs=w_sb[:, lo:hi], start=False, stop=True,
                        )
                    nc.scalar.copy(stg[:, i, :], pp[:, :])

                # km[g,k,d] = K0[g,k,d] * msum[g,d] on GPSIMD (flat-rate;
                # Pool has slack). Issued here — before the previous group's
                # kfold in Pool's in-order queue — so it's ready the moment
                # DVE starts this group's P-mul.
                if not emit_km:
                    return stg, None
                K0 = stg[:, :, OK:OK + NK].rearrange("p g (k d) -> p g k d", k=KV)
                m = stg[:, :, OM:OM + NM]
                km = kmpool.tile([TT, GS, KV, D], f16, name="km")
                nc.gpsimd.tensor_mul(
                    km[:], K0,
                    m.unsqueeze(2).broadcast_to([TT, GS, KV, D]),
                )
                return stg, km

            def phase_b1(g, stg, km, nchunks=1):
                """Group g DVE math: P, fold tree -> attn, C.

                nchunks>1 processes the group in sub-chunks (km computed
                on DVE per chunk) so the first DVE op only waits for the
                first sub-chunk's staging — used for group 0 to cut the
                pipeline-fill latency.
                """
                Q = stg[:, :, OQ:OQ + NQ].rearrange("p g (h d) -> p g h d", h=H)
                Vi = stg[:, :, OV:OV + NV].rearrange("p g (d k) -> p g d k", k=KV)
                K0 = stg[:, :, OK:OK + NK].rearrange("p g (k d) -> p g k d", k=KV)
                m = stg[:, :, OM:OM + NM]

                P = wpool.tile([TT, GS, H, KV, D], f16)
                A1 = wpool.tile([TT, GS * 8, 32], f16)
                A2 = wpool.tile([TT, GS * 8, 16], f16)
                A3 = wpool.tile([TT, GS * 8, 8], f16)
                A4 = wpool.tile([TT, GS * 8, 4], f16)
                A5 = wpool.tile([TT, GS * 8, 2], f16)
                attn = wpool.tile([TT, GS, H, KV], f16)
                C = cpool2.tile([TT, GS, H, D, KV], f16)
                kmd = wpool.tile([TT, GS, KV, D], f16, name="kmd") if km is None else None

                cs = GS // nchunks
                for c in range(nchunks):
                    sl = slice(c * cs, (c + 1) * cs)
                    fl = slice(c * cs * 8, (c + 1) * cs * 8)
                    if km is None:
                        nc.vector.tensor_mul(
                            kmd[:, sl], K0[:, sl],
                            m[:, sl].unsqueeze(2).broadcast_to([TT, cs, KV, D]),
                        )
                    kmc = (kmd if km is None else km)[:, sl]
                    for h in range(H):
                        nc.vector.tensor_mul(
                            P[:, sl, h],
                            Q[:, sl, h].unsqueeze(2).broadcast_to([TT, cs, KV, D]),
                            kmc,
                        )
                    Pf = P[:].rearrange("p g h k d -> p (g h k) d")
                    nc.vector.tensor_add(
                        A1[:, fl], Pf[:, fl, 0:32], Pf[:, fl, 32:64])
                    nc.vector.tensor_add(
                        A2[:, fl], A1[:, fl, 0:16], A1[:, fl, 16:32])
                    nc.vector.tensor_add(
                        A3[:, fl], A2[:, fl, 0:8], A2[:, fl, 8:16])
                    nc.vector.tensor_add(
                        A4[:, fl], A3[:, fl, 0:4], A3[:, fl, 4:8])
                    nc.vector.tensor_add(
                        A5[:, fl], A4[:, fl, 0:2], A4[:, fl, 2:4])
                    nc.vector.tensor_add(
                        attn[:].rearrange("p g h k -> p (g h k)")[:, fl],
                        A5[:, fl, 0], A5[:, fl, 1],
                    )
                    # C[g,h,d,k] = attn[g,h,k] * Vi[g,d,k]  (k innermost: 2x)
                    for h in range(H):
                        nc.vector.tensor_mul(
                            C[:, sl, h],
                            attn[:, sl, h].unsqueeze(2).broadcast_to(
                                [TT, cs, D, KV]),
                            Vi[:, sl],
                        )
                return C

            def phase_b2(g, C):
                """Group g ending: kfold -> xbar transpose -> y = u2@Wo^T.
                Deferred one pipeline step behind phase_b1 so these tail ops
                sit *after* the next groups' staging in every in-order
                engine queue (no head-of-line blocking of the critical
                DVE path)."""
                u = upool.tile([TT, GS, 2 * DIM], f16)
                u2t = u2tpool.tile([DIM, GS * 2, TT], f16)
                ys = yspool.tile([TT, GS, 2, DIM], f32)
                HG = GS // 2
                for half in range(2):
                    hs = half * HG
                    # u[g,h,d] = C[...,0] + C[...,1] (stride-2: flat GPSIMD)
                    uv = u[:, hs:hs + HG].rearrange(
                        "p g (h d) -> p (g h) d", h=H)
                    Cf = C[:, hs:hs + HG].rearrange("p g h d k -> p (g h) d k")
                    nc.gpsimd.tensor_add(uv, Cf[:, :, :, 0], Cf[:, :, :, 1])

                    # Batched u transpose through the DMA xbar: one
                    # instruction block-transposes all HG*2 [128,128] blocks
                    # (no PE transposes, no ACT staging).
                    nc.sync.dma_start_transpose(
                        u2t[:, 2 * hs:2 * (hs + HG), :],
                        u[:, hs:hs + HG].rearrange("p g c -> p (g c)"),
                    )

                    # y = u2 @ Wo^T per tile/j; stage PSUM->SBUF per 4 tiles
                    # (bitcast to uint64 halves the ACT element count)
                    for qq in range(HG // 4):
                        q = half * (HG // 4) + qq
                        yp = yppool.tile([TT, 4, 2, DIM], f32, name="yp", tag="yp")
                        for ii in range(4):
                            for j in range(2):
                                nc.tensor.matmul(
                                    out=yp[:, ii, j, :],
                                    lhsT=u2t[:, (q * 4 + ii) * 2 + j, :],
                                    rhs=wo_sb[:, :],
                                    start=True, stop=True,
                                )
                        nc.scalar.copy(ys[:, q * 4:(q + 1) * 4], yp[:])
                    hbase = (g * GS + hs) * TT
                    dst = y_d[hbase:hbase + HG * TT, :].rearrange(
                        "(i t) (j o) -> t i j o", i=HG, j=2
                    )
                    nc.sync.dma_start(dst, ys[:, hs:hs + HG])

            # Software pipeline: issue group g+1's projection+staging before
            # group g's tail so in-order engine queues never head-of-line
            # block the next group's critical path.
            nxt = phase_a(0, emit_km=False)
            prev_c = None
            for g in range(NG):
                cur = nxt
                if g + 1 < NG:
                    nxt = phase_a(g + 1)
                c = phase_b1(g, *cur, nchunks=(4 if g == 0 else 1))
                if prev_c is not None:
                    phase_b2(g - 1, prev_c)
                prev_c = c
            phase_b2(NG - 1, prev_c)

    nc.compile()
    return nc


def kernel(x, Wq, bq, Wk, bk, Wv, bv, Wo):
    global _COMPILED
    from concourse.bass_utils import run_bass_kernel_spmd

    x = np.asarray(x, dtype=np.float32)
    W_all, b_all, WoT = _fold_weights(
        np.asarray(Wq, np.float32), np.asarray(bq, np.float32),
        np.asarray(Wk, np.float32), np.asarray(bk, np.float32),
        np.asarray(Wv, np.float32), np.asarray(bv, np.float32),
        np.asarray(Wo, np.float32),
    )

    if _COMPILED is None:
        _COMPILED = _build_program()
    nc = _COMPILED

    x2d = x.reshape(T, DIM)
    ones = np.ones((1, TT), dtype=np.float16)
    wall = W_all.astype(np.float16)
    ball = b_all.reshape(1, NPROJ).astype(np.float16)
    woT = WoT.astype(np.float16)
    in_maps = []
    for c in range(NCORES):
        shard = x2d[c * TPC:(c + 1) * TPC]          # [16384, 128]
        in_maps.append({
            "xT": np.ascontiguousarray(shard.T).astype(np.float16),
            "wall": wall,
            "ball": ball,
            "ones": ones,
            "woT": woT,
        })

    res = run_bass_kernel_spmd(nc, in_maps, list(range(NCORES)))
    ys = [res.results[c]["y"] for c in range(NCORES)]
    Y = np.concatenate(ys, axis=0)                   # [131072, 256]
    return Y.reshape(B, 2 * S, DIM)
